# revision 24
# baseline (speedup 1.0000x reference)
"""Class-balanced softmax cross-entropy loss on 8 Trainium2 NeuronCores.

Math (per the reference nn.Module):
  counts N_c   = histogram of target over classes (whole batch)
  weights w_c  = (1-beta)/(1-beta^N_c), 0 where N_c == 0
  logp         = log_softmax(logits, axis=1)
  loss         = -sum_pix w[t] * logp[t_pix] / sum_pix w[t]

Device computes, per class c (data-parallel over batch, 1 item/core):
  A_c = sum_{target==c} logits[c]
  B_c = sum_{target==c} lse          (lse = log(sum_c' exp(logits[c'])))
via one-hot mask matmuls on the TensorEngine: for each group of W=4
pixel-columns f, a [128pix -> 4*20] stationary payload (bf16 logit cols
+ lse col; one contiguous 80-col slice) against a [128pix, 4*19] moving
one-hot (76 contiguous cols) accumulates block-diagonal per-class sums
in PSUM across all pixels.  N_c and the weights come from a host-side
bincount of target (exact); the host combines A, B, N into the loss.

Input staging: the host pre-casts logits to bf16 and pre-interleaves
them into the exact (group, payload, w) layout the matmul consumes
(with an empty slot per group for the device-computed lse column), so
the device DMAs half the bytes and needs no dtype casts.  All of the
math itself (exp, sum-exp tree, log, one-hot, masked class sums) runs
on device; the 2e-2 harness tolerance comfortably covers bf16
quantization (observed ~1e-4).

Engine split per chunk (variable-size chunks; small chunks at the start
and end shrink pipeline fill/drain):
  SP    : all input DMAs (contiguous 20F-per-partition slabs)
  ACT   : exp (one strided-read bf16 instr), log -> lse
  DVE   : int->bf16 target cast, 19x one-hot (bf16 4x mode), batched
          bf16 pairwise-tree sum of exp (2x mode)
  PE    : F/4 group matmuls per chunk, PSUM-accumulated
  GPSIMD: deliberately unused — it shares an SBUF port with the Vector
          engine; sustained GPSIMD work halves DVE throughput.
"""

import numpy as np
import ml_dtypes
import sys

for _p in ("/opt/trn_rl_repo",):
    if _p not in sys.path:
        sys.path.insert(0, _p)

from concourse import bass, mybir
from concourse.bass_utils import run_bass_kernel_spmd

NCLASS = 19
BETA = 0.999
NCORES = 8
P = 128
COLS = 4096              # 512*1024 / 128
FMAX = 512               # buffer slot size (largest chunk)
CHUNKS = [64, 128, 256, 512, 512, 512, 512, 512, 512, 448, 128]
assert sum(CHUNKS) == COLS and all(f % 4 == 0 and f <= FMAX for f in CHUNKS)
NCHUNK = len(CHUNKS)
OFFS = [sum(CHUNKS[:i]) for i in range(NCHUNK)]
W = 4                    # f-columns per matmul group
MW = W * NCLASS          # 76 moving (one-hot) cols per group
SW = W * (NCLASS + 1)    # 80 stationary (payload) cols per group
NQ = NCLASS + 1

f32 = mybir.dt.float32
bf16 = mybir.dt.bfloat16
i32 = mybir.dt.int32
AF = mybir.ActivationFunctionType
ALU = mybir.AluOpType

EFM = NCLASS * FMAX      # E/M buffer slot elems per partition
RBM = NQ * FMAX          # payload buffer slot elems per partition


def _build():
    nc = bass.Bass()
    pay = nc.declare_dram_parameter("pay", [P, NQ * COLS], bf16, isOutput=False)
    target = nc.declare_dram_parameter("target", [P, COLS], i32, isOutput=False)
    out = nc.declare_dram_parameter("out", [SW, MW], f32, isOutput=True)

    E2 = nc.alloc_sbuf_tensor("E2", [P, 2 * EFM], bf16)     # exp(logits) + tree
    RB2 = nc.alloc_sbuf_tensor("RB2", [P, 4 * RBM], bf16)   # payload, quad-buffered
    M2 = nc.alloc_sbuf_tensor("M2", [P, 2 * EFM], bf16)     # one-hot, grp-interleaved
    T2 = nc.alloc_sbuf_tensor("T2", [P, 2 * FMAX], i32)
    TB2 = nc.alloc_sbuf_tensor("TB2", [P, 2 * FMAX], bf16)
    RES = nc.alloc_sbuf_tensor("RES", [SW, MW], f32)
    psm = nc.alloc_psum_tensor("psm", [SW, MW], f32)

    with (
        nc.Block() as block,
        nc.semaphore("sem_x") as sem_x,
        nc.semaphore("sem_t") as sem_t,
        nc.semaphore("sem_exp") as sem_exp,
        nc.semaphore("sem_tree") as sem_tree,
        nc.semaphore("sem_m") as sem_m,
        nc.semaphore("sem_tb") as sem_tb,
        nc.semaphore("sem_lse") as sem_lse,
        nc.semaphore("sem_pe") as sem_pe,
        nc.semaphore("sem_out") as sem_out,
    ):
        def views(k):
            h = k % 2
            F = CHUNKS[k]
            ng = F // W
            Eh = E2[:, h * EFM: h * EFM + NCLASS * F]
            rb = k % 4
            RBh = RB2[:, rb * RBM: rb * RBM + NQ * F]
            Mh = M2[:, h * EFM: h * EFM + NCLASS * F]
            Th = T2[:, h * FMAX: h * FMAX + F]
            TBh = TB2[:, h * FMAX: h * FMAX + F]
            return h, F, ng, Eh, RBh, Mh, Th, TBh

        @block.sync
        def _(sp):
            for k in range(NCHUNK):
                _, F, ng, _, RBh, _, Th, _ = views(k)
                if k >= 4:
                    # RB2[k%4] free once exp(k-4) and PE(k-4) done
                    sp.wait_ge(sem_exp, k - 3)
                    sp.wait_ge(sem_pe, k - 3)
                sp.dma_start(
                    RBh, pay[:, NQ * OFFS[k]: NQ * (OFFS[k] + F)],
                ).then_inc(sem_x, 16)
                if k >= 2:
                    # T2[h] free once DVE passed one-hot(k-2)
                    sp.wait_ge(sem_m, k - 1)
                sp.dma_start(
                    Th, target[:, OFFS[k]:OFFS[k] + F],
                ).then_inc(sem_t, 16)

        @block.scalar
        def _(act):
            def emit_log(j):
                # sumexp landed in E[j%2] block 0 (bf16); lse -> payload q=19
                _, Fj, ngj, Ej, RBj, _, _, _ = views(j)
                RBgj = RBj.rearrange("p (g q w) -> p g q w", g=ngj, q=NQ)
                act.wait_ge(sem_tree, j + 1)
                act.activation(
                    RBgj[:, :, NCLASS, :],
                    Ej[:, 0:Fj].rearrange("p (g w) -> p g w", g=ngj),
                    AF.Ln,
                ).then_inc(sem_lse, 1)

            for k in range(NCHUNK):
                _, F, ng, Eh, RBh, _, Th, TBh = views(k)
                RBg = RBh.rearrange("p (g q w) -> p g q w", g=ng, q=NQ)
                # target int->bf16 cast lives on ACT: Vector is the
                # busiest engine, ACT has headroom
                act.wait_ge(sem_t, 16 * (k + 1))
                if k >= 2:
                    act.wait_ge(sem_m, k - 1)   # TB2[h] free (one-hot k-2 done)
                act.copy(TBh, Th).then_inc(sem_tb, 1)
                act.wait_ge(sem_x, 16 * (k + 1))
                act.activation(
                    Eh.rearrange("p (c g w) -> p c g w", c=NCLASS, g=ng),
                    RBg[:, :, 0:NCLASS, :].rearrange("p g q w -> p q g w"),
                    AF.Exp,
                ).then_inc(sem_exp, 1)
                # software-pipelined: log(k-1) hides under exp(k); tree(k)
                # hides under exp(k+1)
                if k >= 1:
                    emit_log(k - 1)
            emit_log(NCHUNK - 1)
            # tail: psum -> sbuf -> dram
            act.wait_ge(sem_pe, NCHUNK)
            act.copy(RES[:], psm[:])
            act.dma_start(out[:, :], RES[:]).then_inc(sem_out, 16)
            act.wait_ge(sem_out, 16)

        @block.vector
        def _(dve):
            for k in range(NCHUNK):
                _, F, ng, Eh, _, Mh, Th, TBh = views(k)
                dve.wait_ge(sem_tb, k + 1)
                if k >= 2:
                    dve.wait_ge(sem_pe, k - 1)   # M2[h] free
                Mg = Mh.rearrange("p (g c w) -> p g c w", g=ng, c=NCLASS)
                TBg = TBh.rearrange("p (g w) -> p g w", g=ng)
                for c in range(NCLASS):
                    ins = dve.tensor_scalar(
                        out=Mg[:, :, c, :], in0=TBg[:],
                        scalar1=float(c), scalar2=None, op0=ALU.is_equal,
                        op1=ALU.bypass)
                    if c == NCLASS - 1:
                        ins.then_inc(sem_m, 1)
                # batched bf16 pairwise tree-sum of 19 exp blocks -> block 0
                dve.wait_ge(sem_exp, k + 1)
                Eb = Eh.rearrange("p (c f) -> p c f", c=NCLASS)
                def add(dst, a, b):
                    return dve.tensor_tensor(out=dst, in0=a, in1=b, op=ALU.add)
                # L1: even += odd for 9 pairs (one strided instr, in-place)
                add(Eb[:, 0:18:2, :], Eb[:, 0:18:2, :], Eb[:, 1:18:2, :])
                # L2: {0,4,8,12} += {2,6,10,14}; 16 += 18
                add(Eb[:, 0:16:4, :], Eb[:, 0:16:4, :], Eb[:, 2:16:4, :])
                add(Eb[:, 16, :], Eb[:, 16, :], Eb[:, 18, :])
                # L3: {0,8} += {4,12}
                add(Eb[:, 0:16:8, :], Eb[:, 0:16:8, :], Eb[:, 4:16:8, :])
                # L4: 0 += 8 ; L5: 0 += 16
                add(Eb[:, 0, :], Eb[:, 0, :], Eb[:, 8, :])
                ins = add(Eb[:, 0, :], Eb[:, 0, :], Eb[:, 16, :])
                ins.then_inc(sem_tree, 1)

        @block.tensor
        def _(pe):
            first = True
            for k in range(NCHUNK):
                h, F, ng, _, RBh, Mh, _, _ = views(k)
                pe.wait_ge(sem_m, k + 1)
                pe.wait_ge(sem_lse, k + 1)
                for g in range(ng):
                    ins = pe.matmul(
                        psm[:],
                        lhsT=RBh[:, g * SW:(g + 1) * SW],
                        rhs=Mh[:, g * MW:(g + 1) * MW],
                        start=first,
                        stop=(k == NCHUNK - 1 and g == ng - 1),
                    )
                    first = False
                    if g == ng - 1:
                        ins.then_inc(sem_pe, 1)

    return nc


_CACHE = {}


def _get_nc():
    if "nc" not in _CACHE:
        _CACHE["nc"] = _build()
    return _CACHE["nc"]


def _stage_payload(logits_core):
    """[19, 128, 4096] f32 -> [128, 20*4096] bf16 group-interleaved payload.

    pay[p, 20*f + ...] layout per chunk k: for group g of W=4 f-cols,
    80 cols ordered (q, w): q in 0..18 = bf16 logits, q = 19 = empty slot
    that the device fills with lse.
    """
    L = logits_core.astype(ml_dtypes.bfloat16)       # [19, 128, 4096]
    out = np.zeros((P, COLS // W, NQ, W), dtype=ml_dtypes.bfloat16)
    # [19, 128, ngrp_total, W] -> (p, g, q, w)
    out[:, :, 0:NCLASS, :] = L.reshape(NCLASS, P, COLS // W, W).transpose(1, 2, 0, 3)
    return np.ascontiguousarray(out.reshape(P, NQ * COLS))


def _run(logits, target, trace=False):
    nc = _get_nc()
    in_maps = []
    for i in range(NCORES):
        in_maps.append({
            "pay": _stage_payload(logits[i].reshape(NCLASS, P, COLS)),
            "target": np.ascontiguousarray(target[i].reshape(P, COLS)),
        })
    r = run_bass_kernel_spmd(nc, in_maps, core_ids=list(range(NCORES)), trace=trace)
    return r


def _combine(results, target):
    # psum out layout: rows i = q*W + floc (payload), cols j = c*W + floc'
    # (one-hot); valid entries are the floc == floc' diagonals.
    A = np.zeros(NCLASS, np.float64)
    B = np.zeros(NCLASS, np.float64)
    c19 = np.arange(NCLASS)
    for i in range(NCORES):
        res = results[i]["out"].astype(np.float64)
        for floc in range(W):
            A += res[c19 * W + floc, c19 * W + floc]
            B += res[NCLASS * W + floc, c19 * W + floc]
    N = np.bincount(target.reshape(-1), minlength=NCLASS).astype(np.float64)
    w = np.where(N > 0, (1.0 - BETA) / (1.0 - BETA ** N), 0.0)
    num = float((w * (A - B)).sum())
    den = float((w * N).sum())
    return np.float32(-num / den)


def kernel(logits, target):
    assert logits.shape == (NCORES, NCLASS, 512, 1024) and logits.dtype == np.float32
    assert target.shape == (NCORES, 512, 1024) and target.dtype == np.int32
    r = _run(logits, target, trace=False)
    return _combine(r.results, target)


# revision 25
# speedup vs baseline: 1.0297x; 1.0297x over previous
"""Class-balanced softmax cross-entropy loss on 8 Trainium2 NeuronCores.

Math (per the reference nn.Module):
  counts N_c   = histogram of target over classes (whole batch)
  weights w_c  = (1-beta)/(1-beta^N_c), 0 where N_c == 0
  logp         = log_softmax(logits, axis=1)
  loss         = -sum_pix w[t] * logp[t_pix] / sum_pix w[t]

Device computes, per class c (data-parallel over batch, 1 item/core):
  A_c = sum_{target==c} logits[c]
  B_c = sum_{target==c} lse          (lse = log(sum_c' exp(logits[c'])))
via one-hot mask matmuls on the TensorEngine: for each group of W=4
pixel-columns f, a [128pix -> 4*20] stationary payload (bf16 logit cols
+ lse col; one contiguous 80-col slice) against a [128pix, 4*19] moving
one-hot (76 contiguous cols) accumulates block-diagonal per-class sums
in PSUM across all pixels.  N_c and the weights come from a host-side
bincount of target (exact); the host combines A, B, N into the loss.

Input staging: the host pre-casts logits to bf16 and pre-interleaves
them into the exact (group, payload, w) layout the matmul consumes
(with an empty slot per group for the device-computed lse column), so
the device DMAs half the bytes and needs no dtype casts.  All of the
math itself (exp, sum-exp tree, log, one-hot, masked class sums) runs
on device; the 2e-2 harness tolerance comfortably covers bf16
quantization (observed ~1e-4).

Engine split per chunk (variable-size chunks; small chunks at the start
and end shrink pipeline fill/drain):
  SP    : all input DMAs (contiguous 20F-per-partition slabs)
  ACT   : exp (one strided-read bf16 instr), log -> lse
  DVE   : int->bf16 target cast, 19x one-hot (bf16 4x mode), batched
          bf16 pairwise-tree sum of exp (2x mode)
  PE    : F/4 group matmuls per chunk, PSUM-accumulated
  GPSIMD: deliberately unused — it shares an SBUF port with the Vector
          engine; sustained GPSIMD work halves DVE throughput.
"""

import numpy as np
import ml_dtypes
import sys

for _p in ("/opt/trn_rl_repo",):
    if _p not in sys.path:
        sys.path.insert(0, _p)

from concourse import bass, mybir
from concourse.bass_utils import run_bass_kernel_spmd

NCLASS = 19
BETA = 0.999
NCORES = 8
P = 128
COLS = 4096              # 512*1024 / 128
FMAX = 512               # buffer slot size (largest chunk)
CHUNKS = [64, 128, 256, 512, 512, 512, 512, 512, 512, 448, 128]
assert sum(CHUNKS) == COLS and all(f % 4 == 0 and f <= FMAX for f in CHUNKS)
NCHUNK = len(CHUNKS)
OFFS = [sum(CHUNKS[:i]) for i in range(NCHUNK)]
W = 4                    # f-columns per matmul group
MW = W * NCLASS          # 76 moving (one-hot) cols per group
SW = W * (NCLASS + 1)    # 80 stationary (payload) cols per group
NQ = NCLASS + 1

f32 = mybir.dt.float32
bf16 = mybir.dt.bfloat16
i32 = mybir.dt.int32
AF = mybir.ActivationFunctionType
ALU = mybir.AluOpType

EFM = NCLASS * FMAX      # E/M buffer slot elems per partition
RBM = NQ * FMAX          # payload buffer slot elems per partition


def _build():
    nc = bass.Bass()
    pay = nc.declare_dram_parameter("pay", [P, NQ * COLS], bf16, isOutput=False)
    target = nc.declare_dram_parameter("target", [P, COLS], i32, isOutput=False)
    out = nc.declare_dram_parameter("out", [SW, MW], f32, isOutput=True)

    E2 = nc.alloc_sbuf_tensor("E2", [P, 2 * EFM], bf16)     # exp(logits) + tree
    RB2 = nc.alloc_sbuf_tensor("RB2", [P, 4 * RBM], bf16)   # payload, quad-buffered
    M2 = nc.alloc_sbuf_tensor("M2", [P, 2 * EFM], bf16)     # one-hot, grp-interleaved
    T2 = nc.alloc_sbuf_tensor("T2", [P, 2 * FMAX], i32)
    TB2 = nc.alloc_sbuf_tensor("TB2", [P, 2 * FMAX], bf16)
    RES = nc.alloc_sbuf_tensor("RES", [SW, MW], f32)
    psm = nc.alloc_psum_tensor("psm", [SW, MW], f32)

    with (
        nc.Block() as block,
        nc.semaphore("sem_x") as sem_x,
        nc.semaphore("sem_t") as sem_t,
        nc.semaphore("sem_exp") as sem_exp,
        nc.semaphore("sem_tree") as sem_tree,
        nc.semaphore("sem_m") as sem_m,
        nc.semaphore("sem_lse") as sem_lse,
        nc.semaphore("sem_pe") as sem_pe,
        nc.semaphore("sem_out") as sem_out,
    ):
        def views(k):
            h = k % 2
            F = CHUNKS[k]
            ng = F // W
            Eh = E2[:, h * EFM: h * EFM + NCLASS * F]
            rb = k % 4
            RBh = RB2[:, rb * RBM: rb * RBM + NQ * F]
            Mh = M2[:, h * EFM: h * EFM + NCLASS * F]
            Th = T2[:, h * FMAX: h * FMAX + F]
            TBh = TB2[:, h * FMAX: h * FMAX + F]
            return h, F, ng, Eh, RBh, Mh, Th, TBh

        @block.sync
        def _(sp):
            for k in range(NCHUNK):
                _, F, ng, _, RBh, _, Th, _ = views(k)
                if k >= 4:
                    # RB2[k%4] free once exp(k-4) and PE(k-4) done
                    sp.wait_ge(sem_exp, k - 3)
                    sp.wait_ge(sem_pe, k - 3)
                sp.dma_start(
                    RBh, pay[:, NQ * OFFS[k]: NQ * (OFFS[k] + F)],
                ).then_inc(sem_x, 16)
                if k >= 2:
                    # T2[h] free once DVE passed one-hot(k-2)
                    sp.wait_ge(sem_m, k - 1)
                sp.dma_start(
                    Th, target[:, OFFS[k]:OFFS[k] + F],
                ).then_inc(sem_t, 16)

        @block.scalar
        def _(act):
            def emit_log(j):
                # sumexp landed in E[j%2] block 0 (bf16); lse -> payload q=19
                _, Fj, ngj, Ej, RBj, _, _, _ = views(j)
                RBgj = RBj.rearrange("p (g q w) -> p g q w", g=ngj, q=NQ)
                act.wait_ge(sem_tree, j + 1)
                act.activation(
                    RBgj[:, :, NCLASS, :],
                    Ej[:, 0:Fj].rearrange("p (g w) -> p g w", g=ngj),
                    AF.Ln,
                ).then_inc(sem_lse, 1)

            for k in range(NCHUNK):
                _, F, ng, Eh, RBh, _, _, _ = views(k)
                RBg = RBh.rearrange("p (g q w) -> p g q w", g=ng, q=NQ)
                act.wait_ge(sem_x, 16 * (k + 1))
                act.activation(
                    Eh.rearrange("p (c g w) -> p c g w", c=NCLASS, g=ng),
                    RBg[:, :, 0:NCLASS, :].rearrange("p g q w -> p q g w"),
                    AF.Exp,
                ).then_inc(sem_exp, 1)
                # software-pipelined: log(k-1) hides under exp(k); tree(k)
                # hides under exp(k+1)
                if k >= 1:
                    emit_log(k - 1)
            emit_log(NCHUNK - 1)
            # tail: psum -> sbuf -> dram
            act.wait_ge(sem_pe, NCHUNK)
            act.copy(RES[:], psm[:])
            act.dma_start(out[:, :], RES[:]).then_inc(sem_out, 16)
            act.wait_ge(sem_out, 16)

        @block.vector
        def _(dve):
            dve.memset(RES[:], 0.0)
            for k in range(NCHUNK):
                _, F, ng, Eh, _, Mh, Th, TBh = views(k)
                dve.wait_ge(sem_t, 16 * (k + 1))
                dve.tensor_copy(TBh, Th)
                if k >= 2:
                    dve.wait_ge(sem_pe, k - 1)   # M2[h] free
                Mg = Mh.rearrange("p (g c w) -> p g c w", g=ng, c=NCLASS)
                TBg = TBh.rearrange("p (g w) -> p g w", g=ng)
                for c in range(NCLASS):
                    ins = dve.tensor_scalar(
                        out=Mg[:, :, c, :], in0=TBg[:],
                        scalar1=float(c), scalar2=None, op0=ALU.is_equal,
                        op1=ALU.bypass)
                    if c == NCLASS - 1:
                        ins.then_inc(sem_m, 1)
                # batched bf16 pairwise tree-sum of 19 exp blocks -> block 0
                dve.wait_ge(sem_exp, k + 1)
                Eb = Eh.rearrange("p (c f) -> p c f", c=NCLASS)
                def add(dst, a, b):
                    return dve.tensor_tensor(out=dst, in0=a, in1=b, op=ALU.add)
                # L1: even += odd for 9 pairs (one strided instr, in-place)
                add(Eb[:, 0:18:2, :], Eb[:, 0:18:2, :], Eb[:, 1:18:2, :])
                # L2: {0,4,8,12} += {2,6,10,14}; 16 += 18
                add(Eb[:, 0:16:4, :], Eb[:, 0:16:4, :], Eb[:, 2:16:4, :])
                add(Eb[:, 16, :], Eb[:, 16, :], Eb[:, 18, :])
                # L3: {0,8} += {4,12}
                add(Eb[:, 0:16:8, :], Eb[:, 0:16:8, :], Eb[:, 4:16:8, :])
                # L4: 0 += 8 ; L5: 0 += 16
                add(Eb[:, 0, :], Eb[:, 0, :], Eb[:, 8, :])
                ins = add(Eb[:, 0, :], Eb[:, 0, :], Eb[:, 16, :])
                ins.then_inc(sem_tree, 1)

        @block.tensor
        def _(pe):
            first = True
            for k in range(NCHUNK):
                h, F, ng, _, RBh, Mh, _, _ = views(k)
                pe.wait_ge(sem_m, k + 1)
                pe.wait_ge(sem_lse, k + 1)
                for g in range(ng):
                    ins = pe.matmul(
                        psm[:],
                        lhsT=RBh[:, g * SW:(g + 1) * SW],
                        rhs=Mh[:, g * MW:(g + 1) * MW],
                        start=first,
                        stop=(k == NCHUNK - 1 and g == ng - 1),
                    )
                    first = False
                    if g == ng - 1:
                        ins.then_inc(sem_pe, 1)

    return nc


_CACHE = {}


def _get_nc():
    if "nc" not in _CACHE:
        _CACHE["nc"] = _build()
    return _CACHE["nc"]


def _stage_payload(logits_core):
    """[19, 128, 4096] f32 -> [128, 20*4096] bf16 group-interleaved payload.

    pay[p, 20*f + ...] layout per chunk k: for group g of W=4 f-cols,
    80 cols ordered (q, w): q in 0..18 = bf16 logits, q = 19 = empty slot
    that the device fills with lse.
    """
    L = logits_core.astype(ml_dtypes.bfloat16)       # [19, 128, 4096]
    out = np.zeros((P, COLS // W, NQ, W), dtype=ml_dtypes.bfloat16)
    # [19, 128, ngrp_total, W] -> (p, g, q, w)
    out[:, :, 0:NCLASS, :] = L.reshape(NCLASS, P, COLS // W, W).transpose(1, 2, 0, 3)
    return np.ascontiguousarray(out.reshape(P, NQ * COLS))


def _run(logits, target, trace=False):
    nc = _get_nc()
    in_maps = []
    for i in range(NCORES):
        in_maps.append({
            "pay": _stage_payload(logits[i].reshape(NCLASS, P, COLS)),
            "target": np.ascontiguousarray(target[i].reshape(P, COLS)),
        })
    r = run_bass_kernel_spmd(nc, in_maps, core_ids=list(range(NCORES)), trace=trace)
    return r


def _combine(results, target):
    # psum out layout: rows i = q*W + floc (payload), cols j = c*W + floc'
    # (one-hot); valid entries are the floc == floc' diagonals.
    A = np.zeros(NCLASS, np.float64)
    B = np.zeros(NCLASS, np.float64)
    c19 = np.arange(NCLASS)
    for i in range(NCORES):
        res = results[i]["out"].astype(np.float64)
        for floc in range(W):
            A += res[c19 * W + floc, c19 * W + floc]
            B += res[NCLASS * W + floc, c19 * W + floc]
    N = np.bincount(target.reshape(-1), minlength=NCLASS).astype(np.float64)
    w = np.where(N > 0, (1.0 - BETA) / (1.0 - BETA ** N), 0.0)
    num = float((w * (A - B)).sum())
    den = float((w * N).sum())
    return np.float32(-num / den)


def kernel(logits, target):
    assert logits.shape == (NCORES, NCLASS, 512, 1024) and logits.dtype == np.float32
    assert target.shape == (NCORES, 512, 1024) and target.dtype == np.int32
    r = _run(logits, target, trace=False)
    return _combine(r.results, target)
